# revision 1
# baseline (speedup 1.0000x reference)
"""Multi-head attention with fraction-based RoPE ("stoich RoPE") on 8
Trainium2 NeuronCores.

Sharding: each core owns one (batch, query-half) pair — B=4 batches x 2
query halves = 8 shards.  Every core projects Q for its 1024 query rows
and K/V for the full 2048 keys of its batch (K/V projection is computed
on both cores sharing a batch; the 2x redundancy buys a kernel with no
collectives: the attention output rows owned by a core carry the full
head dimension, so the output projection and bias are entirely local).

Per-core device program (SPMD, identical on all 8 cores):
  phase A  per head-pair (8 x 128 dims): project Q^T/K^T/V^T from x^T
           streamed out of DRAM (weights stationary, x moving), add
           biases, apply RoPE to Q/K via precomputed cos/sin tiles and
           32-partition cross-quadrant swaps, PE-transpose V into
           natural layout with a ones column appended (row 64 of the
           P@V' output then carries the softmax denominator).
  phase B  attention per head: scores^T = K^T.T @ Q^T chunks -> exp on
           ACT (scale=1/8 folded in, no max subtraction: |scores/8| is
           O(1) for this operator's input distribution) -> P^T@V'
           accumulation -> reciprocal + K=1 broadcast matmul ->
           normalized attn^T written per pair region.
  phase C  output projection: attn^T chunks stationary, Wo^T moving,
           + bias, DMA out rows.

The host shards/formats inputs (transposes, bias/cos-sin tiles) and
concatenates the 8 output row-shards.
"""

import contextlib
import ctypes
import sys
import types

import numpy as np
import ml_dtypes

import concourse.bass as bass
import concourse.mybir as mybir
import concourse.tile as tile
from concourse.masks import make_identity
from concourse.vector_clock import ScopedClock

# ---------------- problem constants (hardcoded per contract) ----------------
B, T, D = 4, 2048, 1024
H, HD = 16, 64  # heads, head dim
HALF = HD // 2
N_CORES = 8
TQ = T // 2  # query rows per core
P = 128
NQ = 512  # moving-dim tile for matmuls
NPAIR = D // P  # 8 head pairs per core
SCALE = 1.0 / np.sqrt(HD)  # folded into exp()
ROPE_SCALE = 1000.0
ROPE_BASE = 10000.0

F32 = mybir.dt.float32
DT_MM = mybir.dt.bfloat16  # dtype of matmul operands (bfloat16 | float32)

_SO_PATH = "/opt/axon/libaxon_pjrt.so"


# ---------------- axon/NTFF environment shims ----------------
def _ntff_profile_hook():
    try:
        lib = ctypes.CDLL(_SO_PATH)
    except OSError:
        return None
    if not hasattr(lib, "axon_start_nrt_profile"):
        return None
    lib.axon_start_nrt_profile.argtypes = [
        ctypes.POINTER(ctypes.c_int64),
        ctypes.c_size_t,
    ]
    lib.axon_start_nrt_profile.restype = ctypes.c_int64
    lib.axon_stop_nrt_profile.argtypes = [ctypes.c_char_p]
    lib.axon_stop_nrt_profile.restype = ctypes.c_int64

    @contextlib.contextmanager
    def _hook(output_dir, device_ids):
        import jax

        jax.devices()
        if device_ids:
            ids = (ctypes.c_int64 * len(device_ids))(*device_ids)
            rc = lib.axon_start_nrt_profile(ids, len(device_ids))
        else:
            rc = lib.axon_start_nrt_profile(None, 0)
        if rc != 0:
            raise RuntimeError(f"axon_start_nrt_profile rc={rc}")
        try:
            yield
        finally:
            n = lib.axon_stop_nrt_profile(str(output_dir).encode())
            if n < 0:
                raise RuntimeError(f"axon_stop_nrt_profile rc={n}")

    return _hook


def install_shims():
    if "antenv.axon_hooks" not in sys.modules:
        mod = types.ModuleType("antenv.axon_hooks")
        hook = _ntff_profile_hook()
        mod.get_axon_ntff_profile_hook = lambda: hook
        mod.set_axon_ntff_profile_hook = lambda h: None
        sys.modules["antenv.axon_hooks"] = mod
    import concourse.bass_utils as bass_utils

    bass_utils.upload_artifacts = lambda tmpdir: str(tmpdir)

    import os

    if os.environ.get("BASS_LDW_OPT") == "1" and not getattr(
        bass_utils, "_ldw_opt_patched", False
    ):
        orig_run = bass_utils.run_command

        def _run_ldw(argv, **kw):
            argv = [
                "--enable-ldw-opt=true" if a == "--enable-ldw-opt=false" else a
                for a in argv
            ]
            return orig_run(argv, **kw)

        bass_utils.run_command = _run_ldw
        bass_utils._ldw_opt_patched = True


class TileContextSplitDrain(tile.TileContext):
    """This walrus build encodes at most 2 sync waits per CTRL
    instruction; Tile's kernel-tail drain wants one wait per logical
    processor.  Split the waits across single-wait NOPs instead."""

    MAX_WAITS = 1

    def _drain_and_barrier(self, tick_clock, wait_clock):
        nc = self.nc
        carrier = nc.sync.nop(nofuse=True)
        wait_clock.add_sem_waits(
            carrier.ins, ScopedClock({None: tick_clock.global_clock})
        )
        waits = list(carrier.ins.sync_info.on_wait or [])
        if len(waits) > self.MAX_WAITS:
            carrier.ins.sync_info.on_wait[:] = waits[: self.MAX_WAITS]
            for i in range(self.MAX_WAITS, len(waits), self.MAX_WAITS):
                extra = nc.sync.nop(nofuse=True)
                extra.ins.sync_info = mybir.SyncInfo(
                    on_wait=list(waits[i : i + self.MAX_WAITS]), on_update=[]
                )
        nc.sync.drain()
        nc.all_engine_barrier()
        assert self.sems is not None
        popped = nc._tile_sem_poison_stack.pop()
        assert popped is self._sem_poison
        nc.clear_and_free_semaphores(list(self.sems.allocated().values()))
        nc.all_engine_barrier()


def _split_sync_waits(nc, max_waits=1):
    """This walrus build rejects instructions carrying more than ~2 sync
    waits.  Move excess waits onto same-engine NOPs inserted just before
    the instruction (AND semantics are preserved: the engine blocks on
    each carrier in program order)."""
    for f in nc.m.functions:
        for bb in f.blocks:
            out = []
            for inst in bb.instructions:
                si = inst.sync_info
                waits = list(si.on_wait) if si and si.on_wait else []
                if len(waits) > max_waits:
                    for i in range(0, len(waits) - max_waits, max_waits):
                        nop = mybir.InstNoOp(
                            name=nc.get_next_instruction_name(), ins=[], outs=[]
                        )
                        nop.engine = inst.engine
                        nop.sync_info = mybir.SyncInfo(
                            on_wait=list(waits[i : i + max_waits]), on_update=[]
                        )
                        nc.register_instruction(nop, overwrite=True)
                        out.append(nop)
                    si.on_wait[:] = waits[len(waits) - max_waits :]
                out.append(inst)
            bb.instructions[:] = out


# ---------------- device program ----------------
def build_nc(dt_mm=DT_MM):
    nc = bass.Bass(
        "TRN2", target_bir_lowering=False, debug=False, num_devices=N_CORES
    )

    xt = nc.dram_tensor("xt", [D, T], dt_mm, kind="ExternalInput")
    xtq = nc.dram_tensor("xtq", [D, TQ], dt_mm, kind="ExternalInput")
    wqt = nc.dram_tensor("wqt", [D, D], dt_mm, kind="ExternalInput")
    wkt = nc.dram_tensor("wkt", [D, D], dt_mm, kind="ExternalInput")
    wvt = nc.dram_tensor("wvt", [D, D], dt_mm, kind="ExternalInput")
    wot = nc.dram_tensor("wot", [D, D], dt_mm, kind="ExternalInput")
    bq = nc.dram_tensor("bq", [P, NPAIR], F32, kind="ExternalInput")
    bk = nc.dram_tensor("bk", [P, NPAIR], F32, kind="ExternalInput")
    bv = nc.dram_tensor("bv", [P, NPAIR], F32, kind="ExternalInput")
    bob = nc.dram_tensor("bob", [P, D], F32, kind="ExternalInput")
    csaq = nc.dram_tensor("csaq", [P, TQ], dt_mm, kind="ExternalInput")
    csbq = nc.dram_tensor("csbq", [P, TQ], dt_mm, kind="ExternalInput")
    csak = nc.dram_tensor("csak", [P, T], dt_mm, kind="ExternalInput")
    csbk = nc.dram_tensor("csbk", [P, T], dt_mm, kind="ExternalInput")
    out = nc.dram_tensor("out", [TQ, D], F32, kind="ExternalOutput")

    with TileContextSplitDrain(nc) as tc:
        persist_cm = tc.tile_pool(name="persist", bufs=1)
        persist = persist_cm.__enter__()

        def ptile(shape, dt, tag):
            return persist.tile(shape, dt, tag=tag, name=tag)

        with contextlib.ExitStack() as ctx:
            # ---- persistent tiles ----
            csaq_t = ptile([P, TQ], dt_mm, "csaq_t")
            csbq_t = ptile([P, TQ], dt_mm, "csbq_t")
            csak_t = ptile([P, T], dt_mm, "csak_t")
            csbk_t = ptile([P, T], dt_mm, "csbk_t")
            bq_t = ptile([P, NPAIR], F32, "bq_t")
            bk_t = ptile([P, NPAIR], F32, "bk_t")
            bv_t = ptile([P, NPAIR], F32, "bv_t")
            ident = ptile([P, HD], dt_mm, "ident")
            ones64 = ptile([1, HD], mybir.dt.float32r, "ones64")
            ones64_f = ptile([1, HD], F32, "ones64_f")
            attn = [ptile([P, TQ], dt_mm, f"attn{pr}") for pr in range(NPAIR)]
            nc.sync.dma_start(csaq_t[:], csaq[:])
            nc.sync.dma_start(csbq_t[:], csbq[:])
            nc.sync.dma_start(csak_t[:], csak[:])
            nc.sync.dma_start(csbk_t[:], csbk[:])
            nc.sync.dma_start(bq_t[:], bq[:])
            nc.sync.dma_start(bk_t[:], bk[:])
            nc.sync.dma_start(bv_t[:], bv[:])
            make_identity(nc, ident[0:HD, :])
            make_identity(nc, ident[HD : 2 * HD, :])
            nc.vector.memset(ones64_f[:], 1.0)
            with nc.allow_low_precision(reason="ones vector for f32r bcast"):
                nc.scalar.copy(ones64[:], ones64_f[:])

            # ---- pools for the head-pair loop ----
            big = 2 if dt_mm != F32 else 1
            xp = ctx.enter_context(tc.tile_pool(name="xp", bufs=3))
            wp = ctx.enter_context(tc.tile_pool(name="wp", bufs=2))
            rawp = ctx.enter_context(tc.tile_pool(name="rawp", bufs=2))
            ropep = ctx.enter_context(tc.tile_pool(name="ropep", bufs=1))
            vtp = ctx.enter_context(tc.tile_pool(name="vtp", bufs=1))
            qkp = ctx.enter_context(tc.tile_pool(name="qkp", bufs=big))
            vnp = ctx.enter_context(tc.tile_pool(name="vnp", bufs=big))
            exp_p = ctx.enter_context(tc.tile_pool(name="exp_p", bufs=4))
            smallp = ctx.enter_context(tc.tile_pool(name="smallp", bufs=5))
            sumsp = ctx.enter_context(tc.tile_pool(name="sumsp", bufs=1))
            h1p = ctx.enter_context(tc.tile_pool(name="h1p", bufs=2))
            ps_proj = ctx.enter_context(
                tc.tile_pool(name="ps_proj", bufs=2, space="PSUM")
            )
            ps_sc = ctx.enter_context(
                tc.tile_pool(name="ps_sc", bufs=2, space="PSUM")
            )
            ps_po = ctx.enter_context(
                tc.tile_pool(name="ps_po", bufs=2, space="PSUM")
            )

            def rope(raw, ntok, csa_t, csb_t, out_tile):
                # raw/cs/out all dt_mm [P, ntok]
                m1 = ropep.tile([P, T], dt_mm, tag="m1", name="m1")
                m2 = ropep.tile([P, T], dt_mm, tag="m2", name="m2")
                t32 = ropep.tile([32, T], dt_mm, tag="t32", name="t32")
                nc.vector.tensor_mul(m1[:, :ntok], raw[:], csa_t[:, :ntok])
                nc.vector.tensor_mul(m2[:, :ntok], raw[:], csb_t[:, :ntok])
                # swap 32-halves within each 64-block of m2 (in place via t32)
                for blk in range(2):
                    b0 = blk * 64
                    nc.vector.tensor_copy(t32[:, :ntok], m2[b0 : b0 + 32, :ntok])
                    nc.vector.tensor_copy(
                        m2[b0 : b0 + 32, :ntok], m2[b0 + 32 : b0 + 64, :ntok]
                    )
                    nc.vector.tensor_copy(
                        m2[b0 + 32 : b0 + 64, :ntok], t32[:, :ntok]
                    )
                nc.vector.tensor_add(out_tile[:], m1[:, :ntok], m2[:, :ntok])

            def stage_units(pr):
                """Emission units for pair pr's projections + RoPE + V
                transpose.  Each unit emits a small instruction group; the
                attention loop of the previous pair pumps these so the PE
                stays dense while ACT works on exp."""
                d0 = pr * P
                st = {}
                units = []

                def u_wdma():
                    st["wq"] = wp.tile([P, NPAIR, P], dt_mm, tag="wq", name="wq_c")
                    st["wk"] = wp.tile([P, NPAIR, P], dt_mm, tag="wk", name="wk_c")
                    st["wv"] = wp.tile([P, NPAIR, P], dt_mm, tag="wv", name="wv_c")
                    for key, w in (("wq", wqt), ("wk", wkt), ("wv", wvt)):
                        nc.sync.dma_start(
                            st[key][:],
                            w[:, d0 : d0 + P].rearrange("(f p) d -> p f d", p=P),
                        )
                    st["qraw"] = rawp.tile([P, TQ], dt_mm, tag="qraw", name="q_raw")
                    st["kraw"] = rawp.tile([P, T], dt_mm, tag="kraw", name="k_raw")
                    st["vt"] = vtp.tile([P, T], dt_mm, tag="vt", name="v_t")

                units.append(u_wdma)

                def u_xdma(key, nb, src):
                    def go():
                        xc = xp.tile([P, NPAIR, NQ], dt_mm, tag="xc", name="xc")
                        nc.sync.dma_start(
                            xc[:],
                            src[:, nb * NQ : (nb + 1) * NQ].rearrange(
                                "(f p) t -> p f t", p=P
                            ),
                        )
                        st[key] = xc

                    return go

                def u_mm(w_key, x_key, f, start, stop):
                    def go():
                        if start:
                            st["ps"] = ps_proj.tile([P, NQ], F32, tag="ps", name="ps")
                        nc.tensor.matmul(
                            st["ps"][:],
                            st[w_key][:, f, :],
                            st[x_key][:, f, :],
                            start=start,
                            stop=stop,
                        )

                    return go

                def u_evict(b_t, dst_key, dslice):
                    def go():
                        nc.scalar.activation(
                            st[dst_key][:, dslice],
                            st["ps"][:],
                            mybir.ActivationFunctionType.Identity,
                            bias=b_t[:, pr : pr + 1],
                        )

                    return go

                # all DMAs first: deep prefetch so pumped matmuls never
                # wait on HBM
                for nb in range(T // NQ):
                    units.append(u_xdma("x%d" % nb, nb, xt))
                for nb in range(TQ // NQ):
                    units.append(u_xdma("q%d" % nb, nb, xtq))
                for nb in range(T // NQ):
                    for w_key, b_t, dst_key in (("wk", bk_t, "kraw"), ("wv", bv_t, "vt")):
                        for f in range(NPAIR):
                            units.append(
                                u_mm(w_key, "x%d" % nb, f, f == 0, f == NPAIR - 1)
                            )
                        units.append(
                            u_evict(b_t, dst_key, slice(nb * NQ, (nb + 1) * NQ))
                        )
                for nb in range(TQ // NQ):
                    for f in range(NPAIR):
                        units.append(u_mm("wq", "q%d" % nb, f, f == 0, f == NPAIR - 1))
                    units.append(
                        u_evict(bq_t, "qraw", slice(nb * NQ, (nb + 1) * NQ))
                    )

                def u_rope():
                    st["qt"] = qkp.tile([P, TQ], dt_mm, tag="qt", name="qt")
                    rope(st["qraw"], TQ, csaq_t, csbq_t, st["qt"])

                def u_rope2():
                    st["kt"] = qkp.tile([P, T], dt_mm, tag="kt", name="kt")
                    rope(st["kraw"], T, csak_t, csbk_t, st["kt"])

                units.append(u_rope)
                units.append(u_rope2)

                def u_vn_alloc(hh):
                    def go():
                        vn_h = vnp.tile(
                            [P, T // P, P], dt_mm, tag=f"vn{hh}", name="vn_h"
                        )
                        # col 64 = ones (softmax denominator); cols 65.. = zero
                        nc.vector.memset(vn_h[:, :, HD : HD + 1], 1.0)
                        nc.vector.memset(vn_h[:, :, HD + 1 :], 0.0)
                        st[f"vn{hh}"] = vn_h

                    return go

                def u_vtr(hh, ch):
                    def go():
                        tp = ps_proj.tile([P, HD], dt_mm, tag="ps", name="tp")
                        h0 = hh * HD
                        nc.tensor.transpose(
                            tp[:],
                            st["vt"][h0 : h0 + HD, ch * P : (ch + 1) * P],
                            ident[h0 : h0 + HD, :],
                        )
                        nc.vector.tensor_copy(st[f"vn{hh}"][:, ch, :HD], tp[:])

                    return go

                for hh in range(2):
                    units.append(u_vn_alloc(hh))
                    for ch in range(T // P):
                        units.append(u_vtr(hh, ch))
                return st, units

            def pump(units, n):
                for _ in range(n):
                    if units:
                        units.pop(0)()

            def attention(pr, st, next_units, pump_rate):
                """Attention for pair pr using st['qt'/'kt'/'vn*'], pumping
                next pair's units between chunk iterations.  PSUM is evicted
                unnormalized per quarter; one batched reciprocal + four
                broadcast matmuls normalize at pair end."""
                sums = sumsp.tile([1, 4 * NQ], F32, tag="sums", name="sums")
                atn_u = []
                for hh in range(2):
                    h0 = hh * HD
                    for qb in range(TQ // NQ):
                        qs = slice(qb * NQ, (qb + 1) * NQ)
                        seg = hh * 2 + qb
                        po = ps_po.tile([P, NQ], F32, tag="po", name="po")
                        pending_pv = None
                        for ci in range(T // P // 2):
                            ps2 = ps_sc.tile([P, 2 * NQ], F32, tag="sc", name="ps2")
                            for k in range(2):
                                ch = 2 * ci + k
                                nc.tensor.matmul(
                                    ps2[:, k * NQ : (k + 1) * NQ],
                                    st["kt"][h0 : h0 + HD, ch * P : (ch + 1) * P],
                                    st["qt"][h0 : h0 + HD, qs],
                                    start=True,
                                    stop=True,
                                )
                            pexp = exp_p.tile(
                                [P, 2 * NQ], dt_mm, tag="ex", name="pexp"
                            )
                            nc.scalar.activation(
                                pexp[:],
                                ps2[:],
                                mybir.ActivationFunctionType.Exp,
                                scale=float(SCALE),
                            )
                            pump(next_units, pump_rate)
                            # PV runs one iteration behind so exp has a full
                            # iteration of latency to hide
                            if pending_pv is not None:
                                pending_pv()
                            def make_pv(pexp=pexp, ci=ci):
                                def go():
                                    for k in range(2):
                                        ch = 2 * ci + k
                                        nc.tensor.matmul(
                                            po[:],
                                            st[f"vn{hh}"][:, ch, :],
                                            pexp[:, k * NQ : (k + 1) * NQ],
                                            start=(ch == 0),
                                            stop=(ch == T // P - 1),
                                        )
                                return go
                            pending_pv = make_pv()
                        pending_pv()
                        # evict unnormalized: rows 0:64 -> bf16, row 64 -> sums
                        au = smallp.tile([HD, NQ], dt_mm, tag="au", name="au")
                        nc.scalar.copy(au[:], po[:HD, :])
                        nc.vector.tensor_copy(
                            sums[:, seg * NQ : (seg + 1) * NQ],
                            po[HD : HD + 1, :],
                        )
                        atn_u.append(au)
                # batched reciprocal over all four quarter sums
                rec = sumsp.tile(
                    [1, 4 * NQ], mybir.dt.float32r, tag="rec", name="rec"
                )
                with nc.allow_low_precision(reason="recb feeds bf16 attn"):
                    nc.vector.reciprocal(rec[:], sums[:])
                attn_h1 = h1p.tile([HD, TQ], dt_mm, tag="h1", name="attn_h1")
                for hh in range(2):
                    for qb in range(TQ // NQ):
                        qs = slice(qb * NQ, (qb + 1) * NQ)
                        seg = hh * 2 + qb
                        pb = ps_sc.tile([P, NQ], F32, tag="sc", name="pb")
                        nc.tensor.matmul(
                            pb[:HD, :],
                            ones64[:],
                            rec[:, seg * NQ : (seg + 1) * NQ],
                            start=True,
                            stop=True,
                        )
                        recb = smallp.tile([HD, NQ], F32, tag="recb", name="recb")
                        nc.vector.tensor_copy(recb[:], pb[:HD, :])
                        dst = attn[pr] if hh == 0 else attn_h1
                        nc.vector.tensor_mul(
                            dst[:HD, qs], atn_u[seg][:], recb[:]
                        )
                # combine odd head into pair region (cross-quadrant
                # 32-partition copies)
                nc.vector.tensor_copy(attn[pr][64:96, :], attn_h1[0:32, :])
                nc.vector.tensor_copy(attn[pr][96:128, :], attn_h1[32:64, :])

            st, units = stage_units(0)
            pump(units, len(units))
            for pr in range(NPAIR):
                if pr + 1 < NPAIR:
                    nxt_st, nxt_units = stage_units(pr + 1)
                else:
                    nxt_st, nxt_units = None, []
                pump_rate = (len(nxt_units) + 29) // 30 if nxt_units else 0
                attention(pr, st, nxt_units, pump_rate)
                pump(nxt_units, len(nxt_units))
                st = nxt_st

        # ---- output projection (separate pool scope) ----
        with contextlib.ExitStack() as ctx:
            wop = ctx.enter_context(tc.tile_pool(name="wop", bufs=1))
            outp = ctx.enter_context(tc.tile_pool(name="outp", bufs=3))
            ps_o = ctx.enter_context(
                tc.tile_pool(name="ps_o", bufs=4, space="PSUM")
            )
            bob_t = persist.tile([P, D], F32, tag="bob_t", name="bob_t")
            nc.sync.dma_start(bob_t[:], bob[:])
            wo_c = []
            for ch in range(NPAIR):
                wo_ch = wop.tile([P, D], dt_mm, tag=f"wo{ch}", name="wo_ch")
                nc.sync.dma_start(wo_ch[:], wot[ch * P : (ch + 1) * P, :])
                wo_c.append(wo_ch)
            for tb in range(TQ // P):
                ts = slice(tb * P, (tb + 1) * P)
                pout = [
                    ps_o.tile([P, NQ], F32, tag="pout", name="pout")
                    for _ in range(2)
                ]
                for ch in range(NPAIR):
                    for nh in range(2):
                        nc.tensor.matmul(
                            pout[nh][:],
                            attn[ch][:, ts],
                            wo_c[ch][:, nh * NQ : (nh + 1) * NQ],
                            start=(ch == 0),
                            stop=(ch == NPAIR - 1),
                        )
                osb = outp.tile([P, D], F32, tag="osb", name="osb")
                for nh in range(2):
                    nc.vector.tensor_add(
                        osb[:, nh * NQ : (nh + 1) * NQ],
                        pout[nh][:],
                        bob_t[:, nh * NQ : (nh + 1) * NQ],
                    )
                nc.sync.dma_start(out[ts, :], osb[:])

        persist_cm.__exit__(None, None, None)

    _split_sync_waits(nc)
    return nc


# ---------------- host-side input prep ----------------
def _np_dt(dt_mm):
    return ml_dtypes.bfloat16 if dt_mm == mybir.dt.bfloat16 else np.float32


def _cs_tiles(frac_b):
    """csa/csb [128, T] f32 RoPE tiles for one batch (frac_b: [T] f32)."""
    i = np.arange(HALF, dtype=np.float64)
    freq = (ROPE_BASE ** (2.0 * i / HD)).astype(np.float32)  # [32]
    pos = frac_b.astype(np.float32) * np.float32(ROPE_SCALE)
    ang = pos[None, :] / freq[:, None]  # [32, T] f32
    a64 = ang.astype(np.float64)
    cos = np.cos(a64).astype(np.float32)
    sin = np.sin(a64).astype(np.float32)
    csa = np.tile(cos, (4, 1))  # [128, T]
    csb = np.tile(np.concatenate([sin, -sin], axis=0), (2, 1))  # [128, T]
    return np.ascontiguousarray(csa), np.ascontiguousarray(csb)


def make_in_maps(x, frac, Wq, bq, Wk, bk, Wv, bv, Wo, bo, dt_mm=DT_MM):
    npdt = _np_dt(dt_mm)
    wqt = np.ascontiguousarray(Wq.T).astype(npdt)
    wkt = np.ascontiguousarray(Wk.T).astype(npdt)
    wvt = np.ascontiguousarray(Wv.T).astype(npdt)
    wot = np.ascontiguousarray(Wo.T).astype(npdt)
    bq_t = np.ascontiguousarray(bq.reshape(NPAIR, P).T).astype(np.float32)
    bk_t = np.ascontiguousarray(bk.reshape(NPAIR, P).T).astype(np.float32)
    bv_t = np.ascontiguousarray(bv.reshape(NPAIR, P).T).astype(np.float32)
    bob = np.ascontiguousarray(np.tile(bo[None, :], (P, 1))).astype(np.float32)
    in_maps = []
    for c in range(N_CORES):
        b, tqh = c // 2, c % 2
        xt = np.ascontiguousarray(x[b].T).astype(npdt)  # [D, T]
        xtq = np.ascontiguousarray(xt[:, tqh * TQ : (tqh + 1) * TQ])
        csa, csb = _cs_tiles(frac[b])
        in_maps.append(
            {
                "xt": xt,
                "xtq": xtq,
                "wqt": wqt,
                "wkt": wkt,
                "wvt": wvt,
                "wot": wot,
                "bq": bq_t,
                "bk": bk_t,
                "bv": bv_t,
                "bob": bob,
                "csaq": np.ascontiguousarray(
                    csa[:, tqh * TQ : (tqh + 1) * TQ]
                ).astype(npdt),
                "csbq": np.ascontiguousarray(
                    csb[:, tqh * TQ : (tqh + 1) * TQ]
                ).astype(npdt),
                "csak": csa.astype(npdt),
                "csbk": csb.astype(npdt),
            }
        )
    return in_maps


_NC_CACHE = {}


def _get_nc(dt_mm=DT_MM):
    key = str(dt_mm)
    if key not in _NC_CACHE:
        _NC_CACHE[key] = build_nc(dt_mm)
    return _NC_CACHE[key]


def kernel(x, frac, Wq, bq, Wk, bk, Wv, bv, Wo, bo):
    install_shims()
    from concourse.bass_utils import run_bass_kernel_spmd

    x = np.asarray(x, dtype=np.float32)
    frac = np.asarray(frac, dtype=np.float32)
    args = [np.asarray(a, dtype=np.float32) for a in (Wq, bq, Wk, bk, Wv, bv, Wo, bo)]
    in_maps = make_in_maps(x, frac, *args, dt_mm=DT_MM)
    nc = _get_nc(DT_MM)
    res = run_bass_kernel_spmd(nc, in_maps, list(range(N_CORES)))
    out = np.empty((B, T, D), dtype=np.float32)
    for c in range(N_CORES):
        b, tqh = c // 2, c % 2
        out[b, tqh * TQ : (tqh + 1) * TQ, :] = res.results[c]["out"]
    return out



# revision 8
# speedup vs baseline: 1.2238x; 1.2238x over previous
"""Multi-head attention with fraction-based RoPE ("stoich RoPE") on 8
Trainium2 NeuronCores.

Sharding: each core owns one (batch, query-half) pair -- B=4 batches x 2
query halves = 8 shards.  Every core projects Q for its 1024 query rows
and K/V for the full 2048 keys of its batch (2x K/V redundancy buys a
kernel with no collectives).  The host rotates the token order per core
so the core's query tokens are always columns [0, TQ) -- attention is
permutation-invariant over keys as long as K and V share the order.

v2 changes vs the original baseline:
  - x^T resident in SBUF (loaded once, no per-pair re-streaming).
  - scores for the two heads of a pair are issued adjacently: contract
    dim is 64, so the PE runs them CONCURRENTLY in row-tiles (0,0) and
    (64,0) -- 2x score throughput.
  - softmax normalization: per-(head, query-block) reciprocal_approx_fast
    straight off the PSUM denominator row (was: a 12.9us single-lane
    reciprocal that stalled the PE every pair), broadcast by a K=1
    matmul, and the 1/den multiply IS the eviction (kills the separate
    unnormalized-attn eviction on ACT).
  - projection evictions moved from ACT to DVE (ACT does exp only).
  - V transposes grouped 4 chunks per PSUM->SBUF copy.
"""

import contextlib
import ctypes
import sys
import types

import numpy as np
import ml_dtypes

import concourse.bass as bass
import concourse.mybir as mybir
import concourse.tile as tile
from concourse.masks import make_identity
from concourse.vector_clock import ScopedClock

# ---------------- problem constants (hardcoded per contract) ----------------
B, T, D = 4, 2048, 1024
H, HD = 16, 64  # heads, head dim
HALF = HD // 2
N_CORES = 8
TQ = T // 2  # query rows per core
P = 128
NQ = 512  # moving-dim tile for matmuls
NPAIR = D // P  # 8 head pairs per core
NCH = T // P  # 16 key chunks
SCALE = 1.0 / np.sqrt(HD)  # folded into exp()
ROPE_SCALE = 1000.0
ROPE_BASE = 10000.0

F32 = mybir.dt.float32
DT_MM = mybir.dt.bfloat16  # dtype of matmul operands (bfloat16 | float32)

_SO_PATH = "/opt/axon/libaxon_pjrt.so"


# ---------------- axon/NTFF environment shims ----------------
def _ntff_profile_hook():
    try:
        lib = ctypes.CDLL(_SO_PATH)
    except OSError:
        return None
    if not hasattr(lib, "axon_start_nrt_profile"):
        return None
    lib.axon_start_nrt_profile.argtypes = [
        ctypes.POINTER(ctypes.c_int64),
        ctypes.c_size_t,
    ]
    lib.axon_start_nrt_profile.restype = ctypes.c_int64
    lib.axon_stop_nrt_profile.argtypes = [ctypes.c_char_p]
    lib.axon_stop_nrt_profile.restype = ctypes.c_int64

    @contextlib.contextmanager
    def _hook(output_dir, device_ids):
        import jax

        jax.devices()
        if device_ids:
            ids = (ctypes.c_int64 * len(device_ids))(*device_ids)
            rc = lib.axon_start_nrt_profile(ids, len(device_ids))
        else:
            rc = lib.axon_start_nrt_profile(None, 0)
        if rc != 0:
            raise RuntimeError(f"axon_start_nrt_profile rc={rc}")
        try:
            yield
        finally:
            n = lib.axon_stop_nrt_profile(str(output_dir).encode())
            if n < 0:
                raise RuntimeError(f"axon_stop_nrt_profile rc={n}")

    return _hook


def install_shims():
    if "antenv.axon_hooks" not in sys.modules:
        mod = types.ModuleType("antenv.axon_hooks")
        hook = _ntff_profile_hook()
        mod.get_axon_ntff_profile_hook = lambda: hook
        mod.set_axon_ntff_profile_hook = lambda h: None
        sys.modules["antenv.axon_hooks"] = mod
    import concourse.bass_utils as bass_utils

    bass_utils.upload_artifacts = lambda tmpdir: str(tmpdir)

    import os

    if os.environ.get("BASS_LDW_OPT") == "1" and not getattr(
        bass_utils, "_ldw_opt_patched", False
    ):
        orig_run = bass_utils.run_command

        def _run_ldw(argv, **kw):
            argv = [
                "--enable-ldw-opt=true" if a == "--enable-ldw-opt=false" else a
                for a in argv
            ]
            return orig_run(argv, **kw)

        bass_utils.run_command = _run_ldw
        bass_utils._ldw_opt_patched = True


class TileContextSplitDrain(tile.TileContext):
    """This walrus build encodes at most 2 sync waits per CTRL
    instruction; Tile's kernel-tail drain wants one wait per logical
    processor.  Split the waits across single-wait NOPs instead."""

    MAX_WAITS = 1

    def _drain_and_barrier(self, tick_clock, wait_clock):
        nc = self.nc
        carrier = nc.sync.nop(nofuse=True)
        wait_clock.add_sem_waits(
            carrier.ins, ScopedClock({None: tick_clock.global_clock})
        )
        waits = list(carrier.ins.sync_info.on_wait or [])
        if len(waits) > self.MAX_WAITS:
            carrier.ins.sync_info.on_wait[:] = waits[: self.MAX_WAITS]
            for i in range(self.MAX_WAITS, len(waits), self.MAX_WAITS):
                extra = nc.sync.nop(nofuse=True)
                extra.ins.sync_info = mybir.SyncInfo(
                    on_wait=list(waits[i : i + self.MAX_WAITS]), on_update=[]
                )
        nc.sync.drain()
        nc.all_engine_barrier()
        assert self.sems is not None
        popped = nc._tile_sem_poison_stack.pop()
        assert popped is self._sem_poison
        nc.clear_and_free_semaphores(list(self.sems.allocated().values()))
        nc.all_engine_barrier()


def _split_sync_waits(nc, max_waits=1):
    """This walrus build rejects instructions carrying more than ~2 sync
    waits.  Move excess waits onto same-engine NOPs inserted just before
    the instruction (AND semantics are preserved: the engine blocks on
    each carrier in program order)."""
    for f in nc.m.functions:
        for bb in f.blocks:
            out = []
            for inst in bb.instructions:
                si = inst.sync_info
                waits = list(si.on_wait) if si and si.on_wait else []
                if len(waits) > max_waits:
                    for i in range(0, len(waits) - max_waits, max_waits):
                        nop = mybir.InstNoOp(
                            name=nc.get_next_instruction_name(), ins=[], outs=[]
                        )
                        nop.engine = inst.engine
                        nop.sync_info = mybir.SyncInfo(
                            on_wait=list(waits[i : i + max_waits]), on_update=[]
                        )
                        nc.register_instruction(nop, overwrite=True)
                        out.append(nop)
                    si.on_wait[:] = waits[len(waits) - max_waits :]
                out.append(inst)
            bb.instructions[:] = out


# ---------------- device program ----------------
def build_nc(dt_mm=DT_MM):
    nc = bass.Bass(
        "TRN2", target_bir_lowering=False, debug=False, num_devices=N_CORES
    )

    xt = nc.dram_tensor("xt", [D, T], dt_mm, kind="ExternalInput")
    wqt = nc.dram_tensor("wqt", [D, D], dt_mm, kind="ExternalInput")
    wkt = nc.dram_tensor("wkt", [D, D], dt_mm, kind="ExternalInput")
    wvt = nc.dram_tensor("wvt", [D, D], dt_mm, kind="ExternalInput")
    wot = nc.dram_tensor("wot", [D, D], dt_mm, kind="ExternalInput")
    bq = nc.dram_tensor("bq", [P, NPAIR], F32, kind="ExternalInput")
    bk = nc.dram_tensor("bk", [P, NPAIR], F32, kind="ExternalInput")
    bv = nc.dram_tensor("bv", [P, NPAIR], F32, kind="ExternalInput")
    bob = nc.dram_tensor("bob", [P, D], F32, kind="ExternalInput")
    csak = nc.dram_tensor("csak", [P, T], dt_mm, kind="ExternalInput")
    csbk = nc.dram_tensor("csbk", [P, T], dt_mm, kind="ExternalInput")
    out = nc.dram_tensor("out", [TQ, D], F32, kind="ExternalOutput")

    AF = mybir.ActivationFunctionType

    with TileContextSplitDrain(nc) as tc:
        persist_cm = tc.tile_pool(name="persist", bufs=1)
        persist = persist_cm.__enter__()

        def ptile(shape, dt, tag):
            return persist.tile(shape, dt, tag=tag, name=tag)

        with contextlib.ExitStack() as ctx:
            # ---- persistent tiles ----
            xt_t = ptile([P, NPAIR, T], dt_mm, "xt_t")  # resident x^T
            csak_t = ptile([P, T], dt_mm, "csak_t")
            csbk_t = ptile([P, T], dt_mm, "csbk_t")
            bq_t = ptile([P, NPAIR], F32, "bq_t")
            bk_t = ptile([P, NPAIR], F32, "bk_t")
            bv_t = ptile([P, NPAIR], F32, "bv_t")
            ident = ptile([P, HD], dt_mm, "ident")
            ones64_f = ptile([1, HD], F32, "ones64_f")
            attn = [ptile([P, TQ], dt_mm, f"attn{pr}") for pr in range(NPAIR)]
            nc.sync.dma_start(
                xt_t[:], xt[:, :].rearrange("(f p) t -> p f t", p=P)
            )
            nc.sync.dma_start(csak_t[:], csak[:])
            nc.sync.dma_start(csbk_t[:], csbk[:])
            nc.sync.dma_start(bq_t[:], bq[:])
            nc.sync.dma_start(bk_t[:], bk[:])
            nc.sync.dma_start(bv_t[:], bv[:])
            make_identity(nc, ident[0:HD, :])
            make_identity(nc, ident[HD : 2 * HD, :])
            # -1: the Newton chain yields -1/den; the broadcast flips sign
            nc.vector.memset(ones64_f[:], -1.0)

            # ---- pools for the head-pair loop ----
            wp = ctx.enter_context(tc.tile_pool(name="wp", bufs=2))
            rawp = ctx.enter_context(tc.tile_pool(name="rawp", bufs=2))
            ropep = ctx.enter_context(tc.tile_pool(name="ropep", bufs=1))
            vtp = ctx.enter_context(tc.tile_pool(name="vtp", bufs=1))
            qkp = ctx.enter_context(tc.tile_pool(name="qkp", bufs=2))
            vnp = ctx.enter_context(tc.tile_pool(name="vnp", bufs=2))
            exp_p = ctx.enter_context(tc.tile_pool(name="exp_p", bufs=4))
            smallp = ctx.enter_context(tc.tile_pool(name="smallp", bufs=2))
            normp = ctx.enter_context(tc.tile_pool(name="normp", bufs=1))
            h1p = ctx.enter_context(tc.tile_pool(name="h1p", bufs=2))
            ps_proj = ctx.enter_context(
                tc.tile_pool(name="ps_proj", bufs=2, space="PSUM")
            )
            ps_sc = ctx.enter_context(
                tc.tile_pool(name="ps_sc", bufs=2, space="PSUM")
            )
            ps_po = ctx.enter_context(
                tc.tile_pool(name="ps_po", bufs=1, space="PSUM")
            )

            def rope(raw, ntok, csa_t, csb_t, out_tile):
                # raw/cs/out all dt_mm [P, ntok]
                m1 = ropep.tile([P, T], dt_mm, tag="m1", name="m1")
                m2 = ropep.tile([P, T], dt_mm, tag="m2", name="m2")
                t32 = ropep.tile([32, T], dt_mm, tag="t32", name="t32")
                nc.vector.tensor_mul(m1[:, :ntok], raw[:], csa_t[:, :ntok])
                nc.vector.tensor_mul(m2[:, :ntok], raw[:], csb_t[:, :ntok])
                # swap 32-halves within each 64-block of m2 (in place via t32)
                for blk in range(2):
                    b0 = blk * 64
                    nc.vector.tensor_copy(t32[:, :ntok], m2[b0 : b0 + 32, :ntok])
                    nc.vector.tensor_copy(
                        m2[b0 : b0 + 32, :ntok], m2[b0 + 32 : b0 + 64, :ntok]
                    )
                    nc.vector.tensor_copy(
                        m2[b0 + 32 : b0 + 64, :ntok], t32[:, :ntok]
                    )
                nc.vector.tensor_add(out_tile[:], m1[:, :ntok], m2[:, :ntok])

            def stage_units(pr):
                """Emission units for pair pr's projections + RoPE + V
                transpose.  Each unit emits a small instruction group; the
                attention loop of the previous pair pumps these so the PE
                stays dense while ACT works on exp."""
                d0 = pr * P
                st = {}
                units = []

                def u_wdma():
                    st["wq"] = wp.tile([P, NPAIR, P], dt_mm, tag="wq", name="wq_c")
                    st["wk"] = wp.tile([P, NPAIR, P], dt_mm, tag="wk", name="wk_c")
                    st["wv"] = wp.tile([P, NPAIR, P], dt_mm, tag="wv", name="wv_c")
                    for key, w in (("wq", wqt), ("wk", wkt), ("wv", wvt)):
                        nc.sync.dma_start(
                            st[key][:],
                            w[:, d0 : d0 + P].rearrange("(f p) d -> p f d", p=P),
                        )
                    st["qraw"] = rawp.tile([P, TQ], dt_mm, tag="qraw", name="q_raw")
                    st["kraw"] = rawp.tile([P, T], dt_mm, tag="kraw", name="k_raw")
                    st["vt"] = vtp.tile([P, T], dt_mm, tag="vt", name="v_t")

                units.append(u_wdma)

                def u_mm(w_key, nb, f, start, stop):
                    def go():
                        if start:
                            st["ps"] = ps_proj.tile([P, NQ], F32, tag="ps", name="ps")
                        nc.tensor.matmul(
                            st["ps"][:],
                            st[w_key][:, f, :],
                            xt_t[:, f, nb * NQ : (nb + 1) * NQ],
                            start=start,
                            stop=stop,
                        )

                    return go

                def u_evict(b_t, dst_key, dslice):
                    def go():
                        nc.vector.tensor_scalar_add(
                            st[dst_key][:, dslice],
                            st["ps"][:],
                            b_t[:, pr : pr + 1],
                        )

                    return go

                for nb in range(T // NQ):
                    for w_key, b_t, dst_key in (
                        ("wk", bk_t, "kraw"),
                        ("wv", bv_t, "vt"),
                    ):
                        for f in range(NPAIR):
                            units.append(
                                u_mm(w_key, nb, f, f == 0, f == NPAIR - 1)
                            )
                        units.append(
                            u_evict(b_t, dst_key, slice(nb * NQ, (nb + 1) * NQ))
                        )
                for nb in range(TQ // NQ):
                    for f in range(NPAIR):
                        units.append(u_mm("wq", nb, f, f == 0, f == NPAIR - 1))
                    units.append(
                        u_evict(bq_t, "qraw", slice(nb * NQ, (nb + 1) * NQ))
                    )

                def u_rope():
                    st["qt"] = qkp.tile([P, TQ], dt_mm, tag="qt", name="qt")
                    rope(st["qraw"], TQ, csak_t, csbk_t, st["qt"])

                def u_rope2():
                    st["kt"] = qkp.tile([P, T], dt_mm, tag="kt", name="kt")
                    rope(st["kraw"], T, csak_t, csbk_t, st["kt"])

                units.append(u_rope)
                units.append(u_rope2)

                def u_vn_alloc(hh):
                    def go():
                        vn_h = vnp.tile(
                            [P, NCH, HD + 1], dt_mm, tag=f"vn{hh}", name="vn_h"
                        )
                        # col 64 = ones (softmax denominator)
                        nc.vector.memset(vn_h[:, :, HD : HD + 1], 1.0)
                        st[f"vn{hh}"] = vn_h

                    return go

                def u_vtr4(hh, g):
                    # transpose 4 key chunks of head hh, one PSUM->SBUF copy
                    def go():
                        tp = ps_proj.tile([P, 4, HD], dt_mm, tag="ps", name="tp")
                        h0 = hh * HD
                        for k in range(4):
                            ch = g * 4 + k
                            nc.tensor.transpose(
                                tp[:, k, :],
                                st["vt"][h0 : h0 + HD, ch * P : (ch + 1) * P],
                                ident[h0 : h0 + HD, :],
                            )
                        nc.vector.tensor_copy(
                            st[f"vn{hh}"][:, g * 4 : g * 4 + 4, :HD], tp[:]
                        )

                    return go

                for hh in range(2):
                    units.append(u_vn_alloc(hh))
                    for g in range(NCH // 4):
                        units.append(u_vtr4(hh, g))
                return st, units

            def pump(units, n):
                for _ in range(n):
                    if units:
                        units.pop(0)()

            def attention(pr, st, next_units, pump_rate, carry_norm):
                """Attention for pair pr using st['qt'/'kt'/'vn*'].  The two
                heads' score matmuls are issued adjacently (concurrent PE
                row-tiles).  po packs both heads' PV accumulation per
                query-block; normalization is a per-head approx reciprocal
                off the PSUM denominator row + K=1 broadcast matmul + the
                1/den multiply as the eviction.  Returns deferred normalize
                closures for the last query block (run by the caller or the
                next pair's attention)."""
                attn_h1 = h1p.tile([HD, TQ], dt_mm, tag="h1", name="attn_h1")

                def make_norm(po, qb):
                    qs = slice(qb * NQ, (qb + 1) * NQ)
                    # Newton reciprocal of the two heads' denominator rows in
                    # one [1, 2*NQ] pass: bit-trick seed (MAGIC - bits(x),
                    # ~10% err) + two Newton steps via the sign-alternating
                    # form z' = z*(den*z + 2), landing at rec = -1/den.
                    # The broadcast matmul's stationary is -1, flipping sign.
                    den = po[HD : HD + 1, :, :]
                    sd = normp.tile([1, 2, NQ], mybir.dt.int32, tag="sd", name="sd")
                    nc.vector.tensor_scalar(
                        sd[:],
                        den.bitcast(mybir.dt.int32),
                        -1,
                        0x7EF311C3,
                        mybir.AluOpType.mult,
                        mybir.AluOpType.add,
                    )
                    y0 = sd[:].bitcast(F32)
                    t0 = normp.tile([1, 2, NQ], F32, tag="t0", name="t0")
                    nc.vector.tensor_mul(t0[:], den, y0)
                    z1 = normp.tile([1, 2, NQ], F32, tag="z1", name="z1")
                    nc.vector.scalar_tensor_tensor(
                        z1[:], t0[:], -2.0, y0,
                        mybir.AluOpType.add, mybir.AluOpType.mult,
                    )
                    t1 = normp.tile([1, 2, NQ], F32, tag="t1", name="t1")
                    nc.vector.tensor_mul(t1[:], den, z1[:])
                    rec = normp.tile([1, 2, NQ], F32, tag="rec", name="rec")
                    nc.vector.scalar_tensor_tensor(
                        rec[:], t1[:], 2.0, z1[:],
                        mybir.AluOpType.add, mybir.AluOpType.mult,
                    )

                    def bcast_mul():
                        for hh in range(2):
                            pb = ps_sc.tile([HD, NQ], F32, tag="sc", name="pb")
                            nc.tensor.matmul(
                                pb[:],
                                ones64_f[:],
                                rec[0:1, hh, :],
                                start=True,
                                stop=True,
                            )
                            # DVE reads at most one PSUM operand: stage the
                            # broadcast reciprocal in SBUF before the multiply
                            recb = smallp.tile(
                                [HD, NQ], F32, tag=f"recb{hh}", name="recb"
                            )
                            nc.vector.tensor_copy(recb[:], pb[:])
                            dst = attn[pr][0:HD, qs] if hh == 0 else attn_h1[:, qs]
                            nc.vector.tensor_mul(
                                dst, po[0:HD, hh, :], recb[:]
                            )

                    return bcast_mul

                for qb in range(TQ // NQ):
                    qs = slice(qb * NQ, (qb + 1) * NQ)
                    po = ps_po.tile([P, 2, NQ], F32, tag="po", name="po")
                    pending_pv = None
                    for ci in range(NCH):
                        ps2 = ps_sc.tile([P, 2 * NQ], F32, tag="sc", name="ps2")
                        for hh in range(2):
                            h0 = hh * HD
                            nc.tensor.matmul(
                                ps2[:, hh * NQ : (hh + 1) * NQ],
                                st["kt"][h0 : h0 + HD, ci * P : (ci + 1) * P],
                                st["qt"][h0 : h0 + HD, qs],
                                start=True,
                                stop=True,
                            )
                        pexp = exp_p.tile([P, 2 * NQ], dt_mm, tag="ex", name="pexp")
                        nc.scalar.activation(
                            pexp[:], ps2[:], AF.Exp, scale=float(SCALE)
                        )
                        pump(next_units, pump_rate)
                        if ci == 1 and carry_norm is not None:
                            carry_norm()
                            carry_norm = None
                        # PV runs one iteration behind so exp latency is hidden
                        if pending_pv is not None:
                            pending_pv()

                        def make_pv(pexp=pexp, ci=ci, po=po):
                            def go():
                                for hh in range(2):
                                    nc.tensor.matmul(
                                        po[0 : HD + 1, hh, :],
                                        st[f"vn{hh}"][:, ci, :],
                                        pexp[:, hh * NQ : (hh + 1) * NQ],
                                        start=(ci == 0),
                                        stop=(ci == NCH - 1),
                                    )

                            return go

                        pending_pv = make_pv()
                    pending_pv()
                    norm = make_norm(po, qb)
                    if qb < TQ // NQ - 1:
                        carry_norm = norm

                # combine odd head into pair region (cross-quadrant
                # 32-partition copies); norm (last qb) must run first
                def tail():
                    norm()
                    nc.vector.tensor_copy(attn[pr][64:96, :], attn_h1[0:32, :])
                    nc.vector.tensor_copy(attn[pr][96:128, :], attn_h1[32:64, :])

                return tail

            st, units = stage_units(0)
            pump(units, len(units))
            carry_tail = None
            for pr in range(NPAIR):
                if pr + 1 < NPAIR:
                    nxt_st, nxt_units = stage_units(pr + 1)
                else:
                    nxt_st, nxt_units = None, []
                pump_rate = (len(nxt_units) + 29) // 30 if nxt_units else 0
                carry_tail = attention(pr, st, nxt_units, pump_rate, carry_tail)
                pump(nxt_units, len(nxt_units))
                st = nxt_st
            carry_tail()

        # ---- output projection (separate pool scope) ----
        with contextlib.ExitStack() as ctx:
            wop = ctx.enter_context(tc.tile_pool(name="wop", bufs=1))
            outp = ctx.enter_context(tc.tile_pool(name="outp", bufs=3))
            ps_o = ctx.enter_context(
                tc.tile_pool(name="ps_o", bufs=4, space="PSUM")
            )
            bob_t = persist.tile([P, D], F32, tag="bob_t", name="bob_t")
            nc.sync.dma_start(bob_t[:], bob[:])
            wo_c = []
            for ch in range(NPAIR):
                wo_ch = wop.tile([P, D], dt_mm, tag=f"wo{ch}", name="wo_ch")
                nc.sync.dma_start(wo_ch[:], wot[ch * P : (ch + 1) * P, :])
                wo_c.append(wo_ch)
            for tb in range(TQ // P):
                ts = slice(tb * P, (tb + 1) * P)
                pout = [
                    ps_o.tile([P, NQ], F32, tag="pout", name="pout")
                    for _ in range(2)
                ]
                for ch in range(NPAIR):
                    for nh in range(2):
                        nc.tensor.matmul(
                            pout[nh][:],
                            attn[ch][:, ts],
                            wo_c[ch][:, nh * NQ : (nh + 1) * NQ],
                            start=(ch == 0),
                            stop=(ch == NPAIR - 1),
                        )
                osb = outp.tile([P, D], F32, tag="osb", name="osb")
                for nh in range(2):
                    nc.vector.tensor_add(
                        osb[:, nh * NQ : (nh + 1) * NQ],
                        pout[nh][:],
                        bob_t[:, nh * NQ : (nh + 1) * NQ],
                    )
                nc.sync.dma_start(out[ts, :], osb[:])

        persist_cm.__exit__(None, None, None)

    _split_sync_waits(nc)
    return nc


# ---------------- host-side input prep ----------------
def _np_dt(dt_mm):
    return ml_dtypes.bfloat16 if dt_mm == mybir.dt.bfloat16 else np.float32


def _cs_tiles(frac_b):
    """csa/csb [128, T] f32 RoPE tiles for one batch (frac_b: [T] f32)."""
    i = np.arange(HALF, dtype=np.float64)
    freq = (ROPE_BASE ** (2.0 * i / HD)).astype(np.float32)  # [32]
    pos = frac_b.astype(np.float32) * np.float32(ROPE_SCALE)
    ang = pos[None, :] / freq[:, None]  # [32, T] f32
    a64 = ang.astype(np.float64)
    cos = np.cos(a64).astype(np.float32)
    sin = np.sin(a64).astype(np.float32)
    csa = np.tile(cos, (4, 1))  # [128, T]
    csb = np.tile(np.concatenate([sin, -sin], axis=0), (2, 1))  # [128, T]
    return np.ascontiguousarray(csa), np.ascontiguousarray(csb)


def make_in_maps(x, frac, Wq, bq, Wk, bk, Wv, bv, Wo, bo, dt_mm=DT_MM):
    npdt = _np_dt(dt_mm)
    wqt = np.ascontiguousarray(Wq.T).astype(npdt)
    wkt = np.ascontiguousarray(Wk.T).astype(npdt)
    wvt = np.ascontiguousarray(Wv.T).astype(npdt)
    wot = np.ascontiguousarray(Wo.T).astype(npdt)
    bq_t = np.ascontiguousarray(bq.reshape(NPAIR, P).T).astype(np.float32)
    bk_t = np.ascontiguousarray(bk.reshape(NPAIR, P).T).astype(np.float32)
    bv_t = np.ascontiguousarray(bv.reshape(NPAIR, P).T).astype(np.float32)
    bob = np.ascontiguousarray(np.tile(bo[None, :], (P, 1))).astype(np.float32)
    in_maps = []
    for c in range(N_CORES):
        b, tqh = c // 2, c % 2
        # rotate token order so this core's query half is first; attention
        # is permutation-invariant over keys (K and V share the order)
        order = np.concatenate(
            [
                np.arange(tqh * TQ, (tqh + 1) * TQ),
                np.arange((1 - tqh) * TQ, (2 - tqh) * TQ),
            ]
        )
        xt = np.ascontiguousarray(x[b].T[:, order]).astype(npdt)  # [D, T]
        csa, csb = _cs_tiles(frac[b])
        in_maps.append(
            {
                "xt": xt,
                "wqt": wqt,
                "wkt": wkt,
                "wvt": wvt,
                "wot": wot,
                "bq": bq_t,
                "bk": bk_t,
                "bv": bv_t,
                "bob": bob,
                "csak": np.ascontiguousarray(csa[:, order]).astype(npdt),
                "csbk": np.ascontiguousarray(csb[:, order]).astype(npdt),
            }
        )
    return in_maps


_NC_CACHE = {}


def _get_nc(dt_mm=DT_MM):
    key = str(dt_mm)
    if key not in _NC_CACHE:
        _NC_CACHE[key] = build_nc(dt_mm)
    return _NC_CACHE[key]


def kernel(x, frac, Wq, bq, Wk, bk, Wv, bv, Wo, bo):
    install_shims()
    from concourse.bass_utils import run_bass_kernel_spmd

    x = np.asarray(x, dtype=np.float32)
    frac = np.asarray(frac, dtype=np.float32)
    args = [np.asarray(a, dtype=np.float32) for a in (Wq, bq, Wk, bk, Wv, bv, Wo, bo)]
    in_maps = make_in_maps(x, frac, *args, dt_mm=DT_MM)
    nc = _get_nc(DT_MM)
    res = run_bass_kernel_spmd(nc, in_maps, list(range(N_CORES)))
    out = np.empty((B, T, D), dtype=np.float32)
    for c in range(N_CORES):
        b, tqh = c // 2, c % 2
        out[b, tqh * TQ : (tqh + 1) * TQ, :] = res.results[c]["out"]
    return out


# revision 18
# speedup vs baseline: 1.4627x; 1.1952x over previous
"""Multi-head attention with fraction-based RoPE ("stoich RoPE") on 8
Trainium2 NeuronCores.

Sharding: each core owns one (batch, query-half) pair -- B=4 batches x 2
query halves = 8 shards.  Every core projects Q for its 1024 query rows
and K/V for the full 2048 keys of its batch (2x K/V redundancy buys a
kernel with no collectives).  The host rotates the token order per core
so the core's query tokens are always columns [0, TQ) -- attention is
permutation-invariant over keys as long as K and V share the order.

v2 changes vs the original baseline:
  - x^T resident in SBUF (loaded once, no per-pair re-streaming).
  - scores for the two heads of a pair are issued adjacently: contract
    dim is 64, so the PE runs them CONCURRENTLY in row-tiles (0,0) and
    (64,0) -- 2x score throughput.
  - softmax normalization: per-(head, query-block) reciprocal_approx_fast
    straight off the PSUM denominator row (was: a 12.9us single-lane
    reciprocal that stalled the PE every pair), broadcast by a K=1
    matmul, and the 1/den multiply IS the eviction (kills the separate
    unnormalized-attn eviction on ACT).
  - projection evictions moved from ACT to DVE (ACT does exp only).
  - V transposes grouped 4 chunks per PSUM->SBUF copy.
"""

import contextlib
import ctypes
import sys
import types

import numpy as np
import ml_dtypes

import concourse.bass as bass
import concourse.mybir as mybir
import concourse.tile as tile
from concourse.masks import make_identity
from concourse.vector_clock import ScopedClock

# ---------------- problem constants (hardcoded per contract) ----------------
B, T, D = 4, 2048, 1024
H, HD = 16, 64  # heads, head dim
HALF = HD // 2
N_CORES = 8
TQ = T // 2  # query rows per core
P = 128
NQ = 512  # moving-dim tile for matmuls
NPAIR = D // P  # 8 head pairs per core
NCH = T // P  # 16 key chunks
SCALE = 1.0 / np.sqrt(HD)  # folded into exp()
ROPE_SCALE = 1000.0
ROPE_BASE = 10000.0

F32 = mybir.dt.float32
DT_MM = mybir.dt.bfloat16  # dtype of matmul operands (bfloat16 | float32)

_SO_PATH = "/opt/axon/libaxon_pjrt.so"


# ---------------- axon/NTFF environment shims ----------------
def _ntff_profile_hook():
    try:
        lib = ctypes.CDLL(_SO_PATH)
    except OSError:
        return None
    if not hasattr(lib, "axon_start_nrt_profile"):
        return None
    lib.axon_start_nrt_profile.argtypes = [
        ctypes.POINTER(ctypes.c_int64),
        ctypes.c_size_t,
    ]
    lib.axon_start_nrt_profile.restype = ctypes.c_int64
    lib.axon_stop_nrt_profile.argtypes = [ctypes.c_char_p]
    lib.axon_stop_nrt_profile.restype = ctypes.c_int64

    @contextlib.contextmanager
    def _hook(output_dir, device_ids):
        import jax

        jax.devices()
        if device_ids:
            ids = (ctypes.c_int64 * len(device_ids))(*device_ids)
            rc = lib.axon_start_nrt_profile(ids, len(device_ids))
        else:
            rc = lib.axon_start_nrt_profile(None, 0)
        if rc != 0:
            raise RuntimeError(f"axon_start_nrt_profile rc={rc}")
        try:
            yield
        finally:
            n = lib.axon_stop_nrt_profile(str(output_dir).encode())
            if n < 0:
                raise RuntimeError(f"axon_stop_nrt_profile rc={n}")

    return _hook


def install_shims():
    if "antenv.axon_hooks" not in sys.modules:
        mod = types.ModuleType("antenv.axon_hooks")
        hook = _ntff_profile_hook()
        mod.get_axon_ntff_profile_hook = lambda: hook
        mod.set_axon_ntff_profile_hook = lambda h: None
        sys.modules["antenv.axon_hooks"] = mod
    import concourse.bass_utils as bass_utils

    bass_utils.upload_artifacts = lambda tmpdir: str(tmpdir)

    import os

    if os.environ.get("BASS_LDW_OPT") == "1" and not getattr(
        bass_utils, "_ldw_opt_patched", False
    ):
        orig_run = bass_utils.run_command

        def _run_ldw(argv, **kw):
            argv = [
                "--enable-ldw-opt=true" if a == "--enable-ldw-opt=false" else a
                for a in argv
            ]
            return orig_run(argv, **kw)

        bass_utils.run_command = _run_ldw
        bass_utils._ldw_opt_patched = True


class TileContextSplitDrain(tile.TileContext):
    """This walrus build encodes at most 2 sync waits per CTRL
    instruction; Tile's kernel-tail drain wants one wait per logical
    processor.  Split the waits across single-wait NOPs instead."""

    MAX_WAITS = 1

    def _drain_and_barrier(self, tick_clock, wait_clock):
        nc = self.nc
        carrier = nc.sync.nop(nofuse=True)
        wait_clock.add_sem_waits(
            carrier.ins, ScopedClock({None: tick_clock.global_clock})
        )
        waits = list(carrier.ins.sync_info.on_wait or [])
        if len(waits) > self.MAX_WAITS:
            carrier.ins.sync_info.on_wait[:] = waits[: self.MAX_WAITS]
            for i in range(self.MAX_WAITS, len(waits), self.MAX_WAITS):
                extra = nc.sync.nop(nofuse=True)
                extra.ins.sync_info = mybir.SyncInfo(
                    on_wait=list(waits[i : i + self.MAX_WAITS]), on_update=[]
                )
        nc.sync.drain()
        nc.all_engine_barrier()
        assert self.sems is not None
        popped = nc._tile_sem_poison_stack.pop()
        assert popped is self._sem_poison
        nc.clear_and_free_semaphores(list(self.sems.allocated().values()))
        nc.all_engine_barrier()


def _split_sync_waits(nc, max_waits=1):
    """This walrus build rejects instructions carrying more than ~2 sync
    waits.  Move excess waits onto same-engine NOPs inserted just before
    the instruction (AND semantics are preserved: the engine blocks on
    each carrier in program order)."""
    for f in nc.m.functions:
        for bb in f.blocks:
            out = []
            for inst in bb.instructions:
                si = inst.sync_info
                waits = list(si.on_wait) if si and si.on_wait else []
                if len(waits) > max_waits:
                    for i in range(0, len(waits) - max_waits, max_waits):
                        nop = mybir.InstNoOp(
                            name=nc.get_next_instruction_name(), ins=[], outs=[]
                        )
                        nop.engine = inst.engine
                        nop.sync_info = mybir.SyncInfo(
                            on_wait=list(waits[i : i + max_waits]), on_update=[]
                        )
                        nc.register_instruction(nop, overwrite=True)
                        out.append(nop)
                    si.on_wait[:] = waits[len(waits) - max_waits :]
                out.append(inst)
            bb.instructions[:] = out


# ---------------- device program ----------------
def build_nc(dt_mm=DT_MM):
    nc = bass.Bass(
        "TRN2", target_bir_lowering=False, debug=False, num_devices=N_CORES
    )

    xt = nc.dram_tensor("xt", [D, T], dt_mm, kind="ExternalInput")
    wqt = nc.dram_tensor("wqt", [D, D], dt_mm, kind="ExternalInput")
    wkt = nc.dram_tensor("wkt", [D, D], dt_mm, kind="ExternalInput")
    wvt = nc.dram_tensor("wvt", [D, D], dt_mm, kind="ExternalInput")
    wot = nc.dram_tensor("wot", [D, D], dt_mm, kind="ExternalInput")
    bq = nc.dram_tensor("bq", [P, NPAIR], F32, kind="ExternalInput")
    bk = nc.dram_tensor("bk", [P, NPAIR], F32, kind="ExternalInput")
    bv = nc.dram_tensor("bv", [P, NPAIR], F32, kind="ExternalInput")
    bob = nc.dram_tensor("bob", [P, D], F32, kind="ExternalInput")
    csak = nc.dram_tensor("csak", [P, T], dt_mm, kind="ExternalInput")
    csbk = nc.dram_tensor("csbk", [P, T], dt_mm, kind="ExternalInput")
    out = nc.dram_tensor("out", [TQ, D], F32, kind="ExternalOutput")

    AF = mybir.ActivationFunctionType

    with TileContextSplitDrain(nc) as tc:
        persist_cm = tc.tile_pool(name="persist", bufs=1)
        persist = persist_cm.__enter__()

        def ptile(shape, dt, tag):
            return persist.tile(shape, dt, tag=tag, name=tag)

        with contextlib.ExitStack() as ctx:
            # ---- persistent tiles ----
            xt_t = ptile([P, NPAIR, T], dt_mm, "xt_t")  # resident x^T
            csak_t = ptile([P, T], dt_mm, "csak_t")
            csbk_t = ptile([P, T], dt_mm, "csbk_t")
            bq_t = ptile([P, NPAIR], F32, "bq_t")
            bk_t = ptile([P, NPAIR], F32, "bk_t")
            bv_t = ptile([P, NPAIR], F32, "bv_t")
            ident = ptile([P, HD], dt_mm, "ident")
            ones64_f = ptile([1, HD], F32, "ones64_f")
            ones64r = ptile([1, HD], mybir.dt.float32r, "ones64r")
            attn = [ptile([P, TQ], dt_mm, f"attn{pr}") for pr in range(NPAIR)]
            nc.sync.dma_start(
                xt_t[:], xt[:, :].rearrange("(f p) t -> p f t", p=P)
            )
            nc.sync.dma_start(csak_t[:], csak[:])
            nc.sync.dma_start(csbk_t[:], csbk[:])
            nc.sync.dma_start(bq_t[:], bq[:])
            nc.sync.dma_start(bk_t[:], bk[:])
            nc.sync.dma_start(bv_t[:], bv[:])
            make_identity(nc, ident[0:HD, :])
            make_identity(nc, ident[HD : 2 * HD, :])
            # +1: the Newton chain yields -1/den, so pb = -1/den and the
            # attn tiles carry -attn/den; the output projection's
            # (bias - pout) restores the sign
            nc.vector.memset(ones64_f[:], 1.0)
            with nc.allow_low_precision(reason="f32r ones for rec bcast"):
                nc.scalar.copy(ones64r[:], ones64_f[:])

            # ---- pools for the head-pair loop ----
            wp = ctx.enter_context(tc.tile_pool(name="wp", bufs=2))
            rawp = ctx.enter_context(tc.tile_pool(name="rawp", bufs=2))
            ropep = ctx.enter_context(tc.tile_pool(name="ropep", bufs=1))
            vtp = ctx.enter_context(tc.tile_pool(name="vtp", bufs=1))
            qkp = ctx.enter_context(tc.tile_pool(name="qkp", bufs=2))
            vnp = ctx.enter_context(tc.tile_pool(name="vnp", bufs=2))
            exp_p = ctx.enter_context(tc.tile_pool(name="exp_p", bufs=4))
            smallp = ctx.enter_context(tc.tile_pool(name="smallp", bufs=2))
            normp = ctx.enter_context(tc.tile_pool(name="normp", bufs=1))
            h1p = ctx.enter_context(tc.tile_pool(name="h1p", bufs=2))
            ps_proj = ctx.enter_context(
                tc.tile_pool(name="ps_proj", bufs=2, space="PSUM")
            )
            ps_sc = ctx.enter_context(
                tc.tile_pool(name="ps_sc", bufs=2, space="PSUM")
            )
            ps_po = ctx.enter_context(
                tc.tile_pool(name="ps_po", bufs=1, space="PSUM")
            )

            def rope(raw, raws, ntok, csa_t, csb_t, out_tile):
                # out = raw*csa + swap32(raw)*csb.  The 32-block partition
                # swap is done by the DMA engines (raws), freeing the DVE.
                m1 = ropep.tile([P, T], dt_mm, tag="m1", name="m1")
                m2 = ropep.tile([P, T], dt_mm, tag="m2", name="m2")
                nc.vector.tensor_mul(m1[:, :ntok], raw[:], csa_t[:, :ntok])
                nc.vector.tensor_mul(m2[:, :ntok], raws[:], csb_t[:, :ntok])
                nc.vector.tensor_add(out_tile[:], m1[:, :ntok], m2[:, :ntok])

            def dma_swap32(dst, src_t):
                # dst[32-block swapped within each 64-block] = src
                for blk in range(2):
                    b0 = blk * 64
                    nc.sync.dma_start(
                        dst[b0 : b0 + 32, :], src_t[b0 + 32 : b0 + 64, :]
                    )
                    nc.sync.dma_start(
                        dst[b0 + 32 : b0 + 64, :], src_t[b0 : b0 + 32, :]
                    )

            def stage_units(pr):
                """Emission units for pair pr's projections + RoPE + V
                transpose.  Each unit emits a small instruction group; the
                attention loop of the previous pair pumps these so the PE
                stays dense while ACT works on exp."""
                d0 = pr * P
                st = {}
                units = []

                def u_wdma():
                    st["wq"] = wp.tile([P, NPAIR, P], dt_mm, tag="wq", name="wq_c")
                    st["wk"] = wp.tile([P, NPAIR, P], dt_mm, tag="wk", name="wk_c")
                    st["wv"] = wp.tile([P, NPAIR, P], dt_mm, tag="wv", name="wv_c")
                    for key, w in (("wq", wqt), ("wk", wkt), ("wv", wvt)):
                        nc.sync.dma_start(
                            st[key][:],
                            w[:, d0 : d0 + P].rearrange("(f p) d -> p f d", p=P),
                        )
                    st["qraw"] = rawp.tile([P, TQ], dt_mm, tag="qraw", name="q_raw")
                    st["kraw"] = rawp.tile([P, T], dt_mm, tag="kraw", name="k_raw")
                    st["qraws"] = rawp.tile([P, TQ], dt_mm, tag="qraws", name="q_raws")
                    st["kraws"] = rawp.tile([P, T], dt_mm, tag="kraws", name="k_raws")
                    st["vt"] = vtp.tile([P, T], dt_mm, tag="vt", name="v_t")

                units.append(u_wdma)

                def u_mm(w_key, nb, f, start, stop):
                    def go():
                        if start:
                            st["ps"] = ps_proj.tile([P, NQ], F32, tag="ps", name="ps")
                        nc.tensor.matmul(
                            st["ps"][:],
                            st[w_key][:, f, :],
                            xt_t[:, f, nb * NQ : (nb + 1) * NQ],
                            start=start,
                            stop=stop,
                        )

                    return go

                def u_evict(b_t, dst_key, dslice):
                    def go():
                        nc.vector.tensor_scalar_add(
                            st[dst_key][:, dslice],
                            st["ps"][:],
                            b_t[:, pr : pr + 1],
                        )

                    return go

                def u_swap(rkey, skey, nb):
                    def go():
                        sl = slice(nb * NQ, (nb + 1) * NQ)
                        dma_swap32(st[skey][:, sl], st[rkey][:, sl])

                    return go

                for nb in range(T // NQ):
                    for w_key, b_t, dst_key in (
                        ("wk", bk_t, "kraw"),
                        ("wv", bv_t, "vt"),
                    ):
                        for f in range(NPAIR):
                            units.append(
                                u_mm(w_key, nb, f, f == 0, f == NPAIR - 1)
                            )
                        units.append(
                            u_evict(b_t, dst_key, slice(nb * NQ, (nb + 1) * NQ))
                        )
                        if dst_key == "kraw":
                            units.append(u_swap("kraw", "kraws", nb))
                for nb in range(TQ // NQ):
                    for f in range(NPAIR):
                        units.append(u_mm("wq", nb, f, f == 0, f == NPAIR - 1))
                    units.append(
                        u_evict(bq_t, "qraw", slice(nb * NQ, (nb + 1) * NQ))
                    )
                    units.append(u_swap("qraw", "qraws", nb))

                def u_rope():
                    st["qt"] = qkp.tile([P, TQ], dt_mm, tag="qt", name="qt")
                    rope(st["qraw"], st["qraws"], TQ, csak_t, csbk_t, st["qt"])

                def u_rope2():
                    st["kt"] = qkp.tile([P, T], dt_mm, tag="kt", name="kt")
                    rope(st["kraw"], st["kraws"], T, csak_t, csbk_t, st["kt"])

                units.append(u_rope)
                units.append(u_rope2)

                def u_vn_alloc(hh):
                    def go():
                        vn_h = vnp.tile(
                            [P, NCH, HD + 1], dt_mm, tag=f"vn{hh}", name="vn_h"
                        )
                        # col 64 = ones (softmax denominator)
                        nc.vector.memset(vn_h[:, :, HD : HD + 1], 1.0)
                        st[f"vn{hh}"] = vn_h

                    return go

                def u_vtr4(hh, g):
                    # transpose 4 key chunks of head hh, one PSUM->SBUF copy
                    def go():
                        tp = ps_proj.tile([P, 4, HD], dt_mm, tag="ps", name="tp")
                        h0 = hh * HD
                        for k in range(4):
                            ch = g * 4 + k
                            nc.tensor.transpose(
                                tp[:, k, :],
                                st["vt"][h0 : h0 + HD, ch * P : (ch + 1) * P],
                                ident[h0 : h0 + HD, :],
                            )
                        nc.vector.tensor_copy(
                            st[f"vn{hh}"][:, g * 4 : g * 4 + 4, :HD], tp[:]
                        )

                    return go

                for hh in range(2):
                    units.append(u_vn_alloc(hh))
                    for g in range(NCH // 4):
                        units.append(u_vtr4(hh, g))
                return st, units

            def pump(units, n):
                for _ in range(n):
                    if units:
                        units.pop(0)()

            def attention(pr, st, next_units, pump_rate, carry_norm,
                          late_units=None):
                """Attention for pair pr using st['qt'/'kt'/'vn*'].  The two
                heads' score matmuls are issued adjacently (concurrent PE
                row-tiles).  po packs both heads' PV accumulation per
                query-block; normalization is a per-head approx reciprocal
                off the PSUM denominator row + K=1 broadcast matmul + the
                1/den multiply as the eviction.  Returns deferred normalize
                closures for the last query block (run by the caller or the
                next pair's attention)."""
                attn_h1 = h1p.tile([HD, TQ], dt_mm, tag="h1", name="attn_h1")

                def make_norm(po, qb):
                    qs = slice(qb * NQ, (qb + 1) * NQ)
                    # Reciprocal of the two heads' denominator rows in one
                    # [1, 2*NQ] pass: bit-trick seed (MAGIC - bits(x), ~4%
                    # err) + one Newton step z = (den*y0 - 2)*y0 = -1/den
                    # (~0.2% err).  The sign is repaid in the output
                    # projection (osb = bias - pout).  The broadcast across
                    # 64 partitions runs on the idle GpSimd engine.
                    den = po[HD : HD + 1, :, :]
                    sd = normp.tile([1, 2, NQ], mybir.dt.int32, tag="sd", name="sd")
                    nc.vector.tensor_scalar(
                        sd[:],
                        den.bitcast(mybir.dt.int32),
                        -1,
                        0x7EF311C3,
                        mybir.AluOpType.mult,
                        mybir.AluOpType.add,
                    )
                    y0 = sd[:].bitcast(F32)
                    t0 = normp.tile([1, 2, NQ], F32, tag="t0", name="t0")
                    nc.vector.tensor_mul(t0[:], den, y0)
                    rec = normp.tile(
                        [1, 2, NQ], mybir.dt.float32r, tag="rec", name="rec"
                    )
                    with nc.allow_low_precision(reason="f32r rec for bcast mm"):
                        nc.vector.scalar_tensor_tensor(
                            rec[:], t0[:], -2.0, y0,
                            mybir.AluOpType.add, mybir.AluOpType.mult,
                        )

                    def bcast_mul():
                        for hh in range(2):
                            pb = ps_sc.tile([HD, NQ], F32, tag="sc", name="pb")
                            nc.tensor.matmul(
                                pb[:],
                                ones64r[:],
                                rec[0:1, hh, :],
                                start=True,
                                stop=True,
                            )
                            recb = smallp.tile(
                                [HD, NQ], F32, tag=f"recb{hh}", name="recb"
                            )
                            nc.vector.tensor_copy(recb[:], pb[:])
                            dst = attn[pr][0:HD, qs] if hh == 0 else attn_h1[:, qs]
                            nc.vector.tensor_mul(
                                dst, po[0:HD, hh, :], recb[:]
                            )
                        # odd head into the pair region per query block so
                        # attn[pr] completes incrementally (cross-quadrant
                        # 32-partition copies)
                        nc.vector.tensor_copy(
                            attn[pr][64:96, qs], attn_h1[0:32, qs]
                        )
                        nc.vector.tensor_copy(
                            attn[pr][96:128, qs], attn_h1[32:64, qs]
                        )

                    return bcast_mul

                for qb in range(TQ // NQ):
                    qs = slice(qb * NQ, (qb + 1) * NQ)
                    po = ps_po.tile([P, 2, NQ], F32, tag="po", name="po")
                    pending_pv = None
                    for ci in range(NCH):
                        ps2 = ps_sc.tile([P, 2 * NQ], F32, tag="sc", name="ps2")
                        for hh in range(2):
                            h0 = hh * HD
                            nc.tensor.matmul(
                                ps2[:, hh * NQ : (hh + 1) * NQ],
                                st["kt"][h0 : h0 + HD, ci * P : (ci + 1) * P],
                                st["qt"][h0 : h0 + HD, qs],
                                start=True,
                                stop=True,
                            )
                        pexp = exp_p.tile([P, 2 * NQ], dt_mm, tag="ex", name="pexp")
                        nc.scalar.activation(
                            pexp[:], ps2[:], AF.Exp, scale=float(SCALE)
                        )
                        pump(next_units, pump_rate)
                        if ci == 1 and carry_norm is not None:
                            carry_norm()
                            carry_norm = None
                        if late_units and qb == TQ // NQ - 1 and ci >= 2:
                            pump(late_units, 1)
                        # PV runs one iteration behind so exp latency is hidden
                        if pending_pv is not None:
                            pending_pv()

                        def make_pv(pexp=pexp, ci=ci, po=po):
                            def go():
                                for hh in range(2):
                                    nc.tensor.matmul(
                                        po[0 : HD + 1, hh, :],
                                        st[f"vn{hh}"][:, ci, :],
                                        pexp[:, hh * NQ : (hh + 1) * NQ],
                                        start=(ci == 0),
                                        stop=(ci == NCH - 1),
                                    )

                            return go

                        pending_pv = make_pv()
                    pending_pv()
                    norm = make_norm(po, qb)
                    if qb < TQ // NQ - 1:
                        carry_norm = norm

                return norm

            # ---- output projection resources (prefetched up front; the
            # matmuls are pumped into the last pair's attention) ----
            wop = ctx.enter_context(tc.tile_pool(name="wop", bufs=1))
            outp = ctx.enter_context(tc.tile_pool(name="outp", bufs=2))
            bob_t = persist.tile([P, D], F32, tag="bob_t", name="bob_t")
            nc.sync.dma_start(bob_t[:], bob[:])
            wo_c = []
            for ch in range(NPAIR):
                wo_ch = wop.tile([P, D], dt_mm, tag=f"wo{ch}", name="wo_ch")
                nc.sync.dma_start(wo_ch[:], wot[ch * P : (ch + 1) * P, :])
                wo_c.append(wo_ch)

            osb_of = {}

            def u_outproj(tb, nh):
                # half an output row-block: 8 accumulating matmuls + bias-sub
                def go():
                    ts = slice(tb * P, (tb + 1) * P)
                    pout = ps_proj.tile([P, NQ], F32, tag="ps", name="pout")
                    for ch in range(NPAIR):
                        nc.tensor.matmul(
                            pout[:],
                            attn[ch][:, ts],
                            wo_c[ch][:, nh * NQ : (nh + 1) * NQ],
                            start=(ch == 0),
                            stop=(ch == NPAIR - 1),
                        )
                    if tb not in osb_of:
                        osb_of[tb] = outp.tile([P, D], F32, tag="osb", name="osb")
                    osb = osb_of[tb]
                    # attn tiles carry -attn/den (sign from the Newton
                    # chain); bias - pout restores the sign for free
                    nc.vector.tensor_sub(
                        osb[:, nh * NQ : (nh + 1) * NQ],
                        bob_t[:, nh * NQ : (nh + 1) * NQ],
                        pout[:],
                    )
                    if nh == 1:
                        nc.sync.dma_start(out[ts, :], osb[:])

                return go

            st, units = stage_units(0)
            pump(units, len(units))
            carry_tail = None
            # out-proj row-blocks 0..3 read only query-block 0 of attn[7];
            # they are pumped into the last pair's second query block
            late_units = [
                u_outproj(tb, nh) for tb in range(TQ // P // 2) for nh in range(2)
            ]
            for pr in range(NPAIR):
                if pr + 1 < NPAIR:
                    nxt_st, nxt_units = stage_units(pr + 1)
                else:
                    nxt_st, nxt_units = None, []
                pump_rate = (len(nxt_units) + 29) // 30 if nxt_units else 0
                carry_tail = attention(
                    pr, st, nxt_units, pump_rate, carry_tail,
                    late_units=late_units if pr == NPAIR - 1 else None,
                )
                pump(nxt_units, len(nxt_units))
                st = nxt_st
            carry_tail()
            pump(late_units, len(late_units))
            for tb in range(TQ // P // 2, TQ // P):
                for nh in range(2):
                    u_outproj(tb, nh)()

        persist_cm.__exit__(None, None, None)

    _split_sync_waits(nc)
    return nc


# ---------------- host-side input prep ----------------
def _np_dt(dt_mm):
    return ml_dtypes.bfloat16 if dt_mm == mybir.dt.bfloat16 else np.float32


def _cs_tiles(frac_b):
    """csa/csb [128, T] f32 RoPE tiles for one batch (frac_b: [T] f32)."""
    i = np.arange(HALF, dtype=np.float64)
    freq = (ROPE_BASE ** (2.0 * i / HD)).astype(np.float32)  # [32]
    pos = frac_b.astype(np.float32) * np.float32(ROPE_SCALE)
    ang = pos[None, :] / freq[:, None]  # [32, T] f32
    a64 = ang.astype(np.float64)
    cos = np.cos(a64).astype(np.float32)
    sin = np.sin(a64).astype(np.float32)
    csa = np.tile(cos, (4, 1))  # [128, T]
    # csb multiplies the pre-swapped raw (raws[p] = raw[swap32(p)]), so the
    # sign lives at the OUTPUT row: rows 0-31 pair with xr and need -sin,
    # rows 32-63 pair with xl and need +sin
    csb = np.tile(np.concatenate([-sin, sin], axis=0), (2, 1))  # [128, T]
    return np.ascontiguousarray(csa), np.ascontiguousarray(csb)


def make_in_maps(x, frac, Wq, bq, Wk, bk, Wv, bv, Wo, bo, dt_mm=DT_MM):
    npdt = _np_dt(dt_mm)
    wqt = np.ascontiguousarray(Wq.T).astype(npdt)
    wkt = np.ascontiguousarray(Wk.T).astype(npdt)
    wvt = np.ascontiguousarray(Wv.T).astype(npdt)
    wot = np.ascontiguousarray(Wo.T).astype(npdt)
    bq_t = np.ascontiguousarray(bq.reshape(NPAIR, P).T).astype(np.float32)
    bk_t = np.ascontiguousarray(bk.reshape(NPAIR, P).T).astype(np.float32)
    bv_t = np.ascontiguousarray(bv.reshape(NPAIR, P).T).astype(np.float32)
    bob = np.ascontiguousarray(np.tile(bo[None, :], (P, 1))).astype(np.float32)
    in_maps = []
    for c in range(N_CORES):
        b, tqh = c // 2, c % 2
        # rotate token order so this core's query half is first; attention
        # is permutation-invariant over keys (K and V share the order)
        order = np.concatenate(
            [
                np.arange(tqh * TQ, (tqh + 1) * TQ),
                np.arange((1 - tqh) * TQ, (2 - tqh) * TQ),
            ]
        )
        xt = np.ascontiguousarray(x[b].T[:, order]).astype(npdt)  # [D, T]
        csa, csb = _cs_tiles(frac[b])
        in_maps.append(
            {
                "xt": xt,
                "wqt": wqt,
                "wkt": wkt,
                "wvt": wvt,
                "wot": wot,
                "bq": bq_t,
                "bk": bk_t,
                "bv": bv_t,
                "bob": bob,
                "csak": np.ascontiguousarray(csa[:, order]).astype(npdt),
                "csbk": np.ascontiguousarray(csb[:, order]).astype(npdt),
            }
        )
    return in_maps


_NC_CACHE = {}


def _get_nc(dt_mm=DT_MM):
    key = str(dt_mm)
    if key not in _NC_CACHE:
        _NC_CACHE[key] = build_nc(dt_mm)
    return _NC_CACHE[key]


def kernel(x, frac, Wq, bq, Wk, bk, Wv, bv, Wo, bo):
    install_shims()
    from concourse.bass_utils import run_bass_kernel_spmd

    x = np.asarray(x, dtype=np.float32)
    frac = np.asarray(frac, dtype=np.float32)
    args = [np.asarray(a, dtype=np.float32) for a in (Wq, bq, Wk, bk, Wv, bv, Wo, bo)]
    in_maps = make_in_maps(x, frac, *args, dt_mm=DT_MM)
    nc = _get_nc(DT_MM)
    res = run_bass_kernel_spmd(nc, in_maps, list(range(N_CORES)))
    out = np.empty((B, T, D), dtype=np.float32)
    for c in range(N_CORES):
        b, tqh = c // 2, c % 2
        out[b, tqh * TQ : (tqh + 1) * TQ, :] = res.results[c]["out"]
    return out
